# revision 7
# baseline (speedup 1.0000x reference)
"""Trainium2 Bass kernel for DynamicRoutingAggregator.

Math (per batch b):
  shared = tanh(X @ W + b)                        # [T, C*CD], bf16 in SBUF
  A_0 = 0; for it in 0..2:
    Cw = (it==0) ? mask/16 : softmax_c(shared . A_{it}) * mask
    S  = sum_t Cw[t,c] * shared[t,c,:]            # PE junk-matmul [C, C*CD], diag extracted
    V  = squash(S); A_{it+1} = A_{it} + V
  out = V_3

Key identity: logits_k = shared . (sum_{j<k} V_j) for unmasked tokens, so no
logits storage; masked tokens are killed via the Cw mask multiply.

Sharding: data-parallel over batch, 8 batches per core on 8 cores.
Host prep: X -> bf16, transposed to [D, T] per batch (contraction dim on
partitions); W -> bf16; mask -> [128, T/128] f32 chunk layout.
"""

import sys

sys.path.insert(0, "/opt/trn_rl_repo")

import numpy as np
import ml_dtypes

BF = ml_dtypes.bfloat16

B, T, D = 64, 1024, 512
CAPS, CD = 16, 64
U = CAPS * CD  # 1024
NCORES = 8
NB = B // NCORES  # batches per core

_CACHE = {}


def _build(nb, tch, has_bias, opts=None):
    """Build the Bass program for one core: nb batches, tch token-chunks of 128."""
    import concourse.bacc as bacc
    import concourse.bass as bass
    import concourse.tile as tile
    import concourse.mybir as mybir

    opts = opts or {}
    diag_dma = opts.get("diag_dma", True)
    f32 = mybir.dt.float32
    bf16 = mybir.dt.bfloat16
    i32 = mybir.dt.int32
    AF = mybir.ActivationFunctionType
    ALU = mybir.AluOpType
    AX = mybir.AxisListType

    tt = tch * 128  # tokens per batch
    dch = D // 128

    nc = bacc.Bacc("TRN2", num_devices=NCORES)
    xt = nc.declare_dram_parameter("xt", [nb, D, tt], bf16, isOutput=False)
    w = nc.declare_dram_parameter("w", [D, U], bf16, isOutput=False)
    mask = nc.declare_dram_parameter("mask", [nb, 128, tch], f32, isOutput=False)
    if has_bias:
        bbc = nc.declare_dram_parameter("bbc", [128, U], f32, isOutput=False)
    out = nc.declare_dram_parameter("out", [nb, U], f32, isOutput=True)

    with tile.TileContext(nc) as tc:
        with (
            tc.tile_pool(name="wp", bufs=1) as wp,
            tc.tile_pool(name="xp", bufs=3) as xp,
            tc.tile_pool(name="mp", bufs=3) as mp,
            tc.tile_pool(name="shp", bufs=2) as shp,
            tc.tile_pool(name="cwp", bufs=10) as cwp,
            tc.tile_pool(name="prp", bufs=4) as prp,
            tc.tile_pool(name="lgp", bufs=8) as lgp,
            tc.tile_pool(name="smp", bufs=8) as smp,
            tc.tile_pool(name="abp", bufs=3) as abp,
            tc.tile_pool(name="mmps", bufs=2, space="PSUM") as mmps,
            tc.tile_pool(name="sps", bufs=1, space="PSUM") as sps,
            tc.tile_pool(name="bcps", bufs=1, space="PSUM") as bcps,
        ):
            ones_bf = wp.tile([1, 128], bf16)
            nc.vector.memset(ones_bf[:], 1.0)
            w_sb = wp.tile([128, dch * U], bf16)
            for j in range(dch):
                nc.sync.dma_start(w_sb[:, j * U:(j + 1) * U], w[j * 128:(j + 1) * 128, :])
            if has_bias:
                bb_sb = wp.tile([128, U], f32)
                nc.sync.dma_start(bb_sb[:], bbc[:, :])

            for bi in range(nb):
                xt_sb = xp.tile([128, dch * tt], bf16)
                for j in range(dch):
                    nc.sync.dma_start(
                        xt_sb[:, j * tt:(j + 1) * tt], xt[bi, j * 128:(j + 1) * 128, :]
                    )
                mk_sb = mp.tile([128, tch], f32)
                nc.sync.dma_start(mk_sb[:], mask[bi])

                # ---- shared = tanh(X W + b), one [128, U] chunk of tokens at a time
                shared = shp.tile([128, tch * U], bf16)
                for tci in range(tch):
                    ps = mmps.tile([128, U], f32)
                    for j in range(dch):
                        lhsT = xt_sb[:, j * tt + tci * 128: j * tt + (tci + 1) * 128]
                        for nh in range(2):
                            nc.tensor.matmul(
                                ps[:, nh * 512:(nh + 1) * 512],
                                lhsT=lhsT,
                                rhs=w_sb[:, j * U + nh * 512: j * U + nh * 512 + 512],
                                start=(j == 0),
                                stop=(j == dch - 1),
                            )
                    if has_bias:
                        nc.vector.tensor_add(ps[:], ps[:], bb_sb[:])
                    nc.scalar.activation(shared[:, tci * U:(tci + 1) * U], ps[:], AF.Tanh)

                # ---- 3 routing iterations
                A = None
                abc = None
                for it in range(3):
                    s_ps = sps.tile([CAPS, U], f32)
                    for tci in range(tch):
                        sh_sl = shared[:, tci * U:(tci + 1) * U]
                        if it == 0:
                            cw = cwp.tile([128, CAPS], bf16)
                            nc.vector.tensor_scalar_mul(
                                cw[:],
                                mk_sb[:, tci:tci + 1].broadcast_to((128, CAPS)),
                                1.0 / 16.0,
                            )
                        else:
                            prod = prp.tile([128, U], bf16)
                            nc.vector.tensor_mul(prod[:], sh_sl, abc[:])
                            lg = lgp.tile([128, CAPS], f32)
                            nc.vector.tensor_reduce(
                                lg[:],
                                prod[:].rearrange("p (c d) -> p c d", d=CD),
                                axis=AX.X,
                                op=ALU.add,
                            )
                            eo = lgp.tile([128, CAPS], f32)
                            se = lgp.tile([128, 1], f32)
                            nc.scalar.activation(eo[:], lg[:], AF.Exp, accum_out=se[:])
                            rc = lgp.tile([128, 1], f32)
                            nc.vector.reciprocal(rc[:], se[:])
                            cw = cwp.tile([128, CAPS], bf16)
                            nc.vector.tensor_scalar(
                                cw[:], eo[:], rc[:], mk_sb[:, tci:tci + 1],
                                op0=ALU.mult, op1=ALU.mult,
                            )
                        for nh in range(2):
                            nc.tensor.matmul(
                                s_ps[:, nh * 512:(nh + 1) * 512],
                                lhsT=cw[:],
                                rhs=sh_sl[:, nh * 512:(nh + 1) * 512],
                                start=(tci == 0),
                                stop=(tci == tch - 1),
                            )

                    # ---- diag extract: S[c, :] = s_ps[c, c*CD:(c+1)*CD]
                    S = smp.tile([CAPS, CD], f32)
                    if diag_dma:
                        ssb = smp.tile([CAPS, U], f32)
                        nc.scalar.copy(ssb[:], s_ps[:])
                        src = bass.AP(ssb[:].tensor, ssb[:].offset,
                                      [[U + CD, CAPS], [1, CD]])
                        nc.sync.dma_start(S[:], src)
                    else:
                        for c in range(CAPS):
                            nc.scalar.copy(S[c:c + 1, :], s_ps[c:c + 1, c * CD:(c + 1) * CD])

                    # ---- squash: V = (q/(1+q)) * S / sqrt(q + 1e-8), q = |S|^2
                    sq = smp.tile([CAPS, CD], f32)
                    q = smp.tile([CAPS, 1], f32)
                    nc.vector.tensor_mul(sq[:], S[:], S[:])
                    nc.vector.tensor_reduce(q[:], sq[:], axis=AX.X, op=ALU.add)
                    nc.vector.tensor_scalar_add(q[:], q[:], 1e-8)
                    # rsqrt via bit trick + 2 Newton steps
                    y = smp.tile([CAPS, 1], f32)
                    t1 = smp.tile([CAPS, 1], f32)
                    nc.vector.tensor_scalar(
                        t1[:].bitcast(i32), q[:].bitcast(i32), 1, None,
                        op0=ALU.logical_shift_right,
                    )
                    nc.vector.tensor_scalar(
                        t1[:].bitcast(i32), t1[:].bitcast(i32), -1, None,
                        op0=ALU.bitwise_xor,
                    )
                    nc.vector.tensor_scalar(
                        y[:].bitcast(i32), t1[:].bitcast(i32), 0x5F3759E0, None,
                        op0=ALU.add,
                    )
                    for _ in range(2):
                        t2 = smp.tile([CAPS, 1], f32)
                        nc.vector.tensor_mul(t2[:], y[:], y[:])
                        nc.vector.tensor_mul(t2[:], t2[:], q[:])
                        nc.vector.tensor_scalar(t2[:], t2[:], -0.5, 1.5,
                                                op0=ALU.mult, op1=ALU.add)
                        nc.vector.tensor_mul(y[:], y[:], t2[:])
                    f = smp.tile([CAPS, 1], f32)
                    u = smp.tile([CAPS, 1], f32)
                    nc.vector.tensor_mul(f[:], q[:], y[:])
                    nc.vector.tensor_scalar_add(u[:], q[:], 1.0)
                    nc.vector.reciprocal(u[:], u[:])
                    nc.vector.tensor_mul(f[:], f[:], u[:])
                    V = smp.tile([CAPS, CD], f32)
                    nc.vector.tensor_scalar_mul(V[:], S[:], f[:])

                    if it == 0:
                        A = V
                    elif it == 1:
                        A2 = smp.tile([CAPS, CD], f32)
                        nc.vector.tensor_add(A2[:], A[:], V[:])
                        A = A2

                    if it < 2:
                        # A -> bf16 row -> broadcast to [128, U]
                        abf = smp.tile([CAPS, CD], bf16)
                        nc.vector.tensor_copy(abf[:], A[:])
                        arow = abp.tile([1, U], bf16)
                        nc.sync.dma_start(arow[0:1, :], abf[:])
                        bc_ps = bcps.tile([128, U], f32)
                        for nh in range(2):
                            nc.tensor.matmul(
                                bc_ps[:, nh * 512:(nh + 1) * 512],
                                lhsT=ones_bf[:],
                                rhs=arow[0:1, nh * 512:(nh + 1) * 512],
                                start=True, stop=True,
                            )
                        abc = abp.tile([128, U], bf16)
                        nc.scalar.copy(abc[:], bc_ps[:])
                    else:
                        nc.sync.dma_start(out[bi:bi + 1, :], V[:])

    nc.compile()
    return nc


def _get_nc(nb, tch, has_bias, opts=None):
    key = (nb, tch, has_bias, tuple(sorted((opts or {}).items())))
    if key not in _CACHE:
        _CACHE[key] = _build(nb, tch, has_bias, opts)
    return _CACHE[key]


def _prep_core_inputs(X, mask, W, b, nb, tch):
    """Host-side prep for one core's slice. X [nb,tt,D] f32 -> dict of arrays."""
    tt = tch * 128
    assert X.shape[1] == tt and mask.shape[1] == tt
    xt = np.ascontiguousarray(
        X.astype(BF).transpose(0, 2, 1)
    )  # [nb, D, tt] bf16
    mk = np.ascontiguousarray(
        mask.astype(np.float32).reshape(nb, tch, 128).transpose(0, 2, 1)
    )  # [nb, 128, tch]
    d = {"xt": xt, "mask": mk}
    return d


COMPACT_TCH = 5  # 640 token slots; batches with more surviving tokens fall back


def _compact(X, mask, tt):
    """Keep only unmasked tokens, zero-pad to tt. Returns (Xc, maskc) or None."""
    Bn = X.shape[0]
    Xc = np.zeros((Bn, tt, X.shape[2]), np.float32)
    mc = np.zeros((Bn, tt), np.int32)
    for i in range(Bn):
        idx = np.flatnonzero(mask[i])
        if len(idx) > tt:
            return None
        Xc[i, :len(idx)] = X[i, idx]
        mc[i, :len(idx)] = 1
    return Xc, mc


def kernel(input_tensors, mask, W, b):
    input_tensors = np.asarray(input_tensors, dtype=np.float32)
    mask = np.asarray(mask)
    W = np.asarray(W, dtype=np.float32)
    b = np.asarray(b, dtype=np.float32)

    has_bias = bool(np.any(b != 0.0))
    comp = _compact(input_tensors, mask, COMPACT_TCH * 128)
    if comp is not None:
        input_tensors, mask = comp
        tch = COMPACT_TCH
    else:
        tch = T // 128
    nc = _get_nc(NB, tch, has_bias)

    wb = np.ascontiguousarray(W.astype(BF))  # [D, U] bf16
    in_maps = []
    for core in range(NCORES):
        sl = slice(core * NB, (core + 1) * NB)
        d = _prep_core_inputs(input_tensors[sl], mask[sl], W, b, NB, tch)
        d["w"] = wb
        if has_bias:
            d["bbc"] = np.broadcast_to(b.astype(np.float32), (128, U)).copy()
        in_maps.append(d)

    from concourse.bass_utils import run_bass_kernel_spmd

    res = run_bass_kernel_spmd(nc, in_maps, list(range(NCORES)))
    out = np.concatenate([np.asarray(res.results[i]["out"]) for i in range(NCORES)], 0)
    return out.astype(np.float32)


if __name__ == "__main__":
    rng = np.random.default_rng(0)
    X = rng.standard_normal((B, T, D), dtype=np.float32)
    mk = rng.integers(0, 2, (B, T)).astype(np.int32)
    Wm = (rng.standard_normal((D, U), dtype=np.float32) / np.sqrt(D)).astype(np.float32)
    bv = np.zeros((U,), np.float32)
    o = kernel(X, mk, Wm, bv)
    print("out", o.shape, o.dtype, np.abs(o).max())
